# revision 1
# baseline (speedup 1.0000x reference)
"""AttentionBlock (GroupNorm + 8-head self-attention + out-proj + residual) on 8 trn2 cores.

Sharding: core = (batch b, query-half ih).  Each core gets x[b] rolled so that
"its" 1024 query positions are columns 0:1024; K/V are computed over the full
(rolled) L=2048, which is sound because attention and the group-norm statistics
are invariant to a permutation of key/value positions.  Output is the core's
[512, 1024] slice of proj + residual; the host reassembles [4, 512, 2048].

Performance structure (v2):
 - The softmax exp on the ACT engine is the hard floor (~16.8M elements/core,
   ~1.25us per [128,1024] exp measured on HW), so the kernel is built as one
   software-pipelined stream that keeps ACT streaming: per pipeline unit
   u=(pair, ib, jcpair, head) the PE emits the S-matmuls of unit u+1 BEFORE
   the O-matmul of unit u, so the in-order PE queue never blocks exp.
 - exp writes float8e4m3 directly (scale=1/8, bias=-1.5 so max exp ~163 stays
   under e4m3's 240; the bias cancels in softmax).  V is stored fp8 with
   jc-pair interleaving so the O accumulation runs in DoubleRow fp8 perf mode
   (2x contraction depth per instruction; dual-fp8 Ldweights needs the
   stationary block width to be a multiple of 32, hence the 96-wide pad).
 - Softmax denominator comes free as a ones-column in the O matmul; per-head
   normalization = PSUM->SBUF copy + bf16 reciprocal + 1-partition broadcast
   matmul + DVE multiply.
 - qkv and out-proj run in bf16 (fp8 DoubleRow for these measured SLOWER on
   HW end-to-end despite a 2x standalone advantage — the extra PE occupancy
   keeps the tensor engine's pstate high for the latency-critical S matmuls)
   and are interleaved into PE slack behind the stream as deadline-scheduled
   "filler" units.
 - Startup: one DMA per 128-row tile (HWDGE charges ~500ns fixed per DMACopy
   on one shared device), group-norm stats split DVE (tiles 0-2, bn_stats) /
   ACT (tile 3 via Copy/Square accumulate — same act table set as Exp), and
   rsqrt via a DVE-only Newton iteration so the Exp table loads exactly once.
"""

import sys

sys.path.insert(0, "/opt/trn_rl_repo")

import numpy as np
import ml_dtypes

import concourse.bass as bass
import concourse.mybir as mybir
import concourse.tile as tile
from concourse import bacc
from concourse.vector_clock import ScopedClock, VectorClock
from concourse.bass_utils import run_bass_kernel_spmd

F32 = mybir.dt.float32
F32R = mybir.dt.float32r
BF16 = mybir.dt.bfloat16
FP8 = mybir.dt.float8e4
AX = mybir.AxisListType
OP = mybir.AluOpType
ACTF = mybir.ActivationFunctionType
DR = mybir.MatmulPerfMode.DoubleRow

B, C, L = 4, 512, 2048
H, D = 8, 64
G, EPS = 32, 1e-5
LQ = L // 2          # queries per core
CT = C // 128        # channel tiles
NJC = L // 128       # key chunks of 128
NJP = NJC // 2       # key chunk pairs (DoubleRow granularity)
NIB = LQ // 512      # 512-wide query blocks
D1 = 96              # V^T block: 64 values + ones col + 31 pad cols; dual-fp8
                     # Ldweights requires the output-partition width to be a
                     # multiple of 32.  PSUM rows 65..95 accumulate ignored
                     # garbage; DR matmul cost depends only on moving columns.
EXP_BIAS = -1.5      # exp(S/8 - 1.5): keeps max under e4m3's 240, cancels in softmax


class _SplitDrainTC(tile.TileContext):
    """Stock exit puts every outstanding proc's wait on one SP Drain; this
    walrus build caps sync-waits per instruction, so spread them over
    single-wait NOPs first."""

    def _drain_and_barrier(self, tick_clock, wait_clock):
        g = tick_clock.global_clock
        for proc in range(len(g)):
            if g[proc] == 0:
                continue
            vc = VectorClock([0] * len(g))
            vc.require_at_least(proc, g[proc])
            nop = self.nc.sync.nop(hint=f"split_drain_{proc}")
            wait_clock.add_sem_waits(nop.ins, ScopedClock({None: vc}))
        self.nc.sync.drain()
        self.nc.all_engine_barrier()
        assert self.sems is not None
        popped = self.nc._tile_sem_poison_stack.pop()
        assert popped is self._sem_poison
        self.nc.clear_and_free_semaphores(list(self.sems.allocated().values()))
        self.nc.all_engine_barrier()


def build_nc(reps: int = 1):
    nc = bacc.Bacc("TRN2", target_bir_lowering=False, num_devices=8)

    xd = nc.declare_dram_parameter("x", [C, L], BF16, isOutput=False)
    wqkvT = nc.declare_dram_parameter("wqkvT", [C, 3 * C], BF16, isOutput=False)
    woutT = nc.declare_dram_parameter("woutT", [C, C], BF16, isOutput=False)
    gnwd = nc.declare_dram_parameter("gnw", [CT, 128], F32, isOutput=False)
    gnbd = nc.declare_dram_parameter("gnb", [CT, 128], F32, isOutput=False)
    boutd = nc.declare_dram_parameter("bout", [128, CT], F32, isOutput=False)
    identd = nc.declare_dram_parameter("ident", [128, 128], F32, isOutput=False)
    yd = nc.declare_dram_parameter("y", [C, LQ], BF16, isOutput=True)

    import contextlib

    with _SplitDrainTC(nc) as tc:
        with (
            tc.For_i(0, reps, 1) if reps > 1 else contextlib.nullcontext()
        ), tc.tile_pool(name="persist", bufs=1) as pp:
            x_tiles = [pp.tile([128, L], BF16, name=f"x{t}", tag=f"x{t}") for t in range(CT)]
            wq_all = pp.tile([128, CT, 3 * C], BF16, name="wq", tag="wq")
            wo_all = pp.tile([128, CT, C], BF16, name="wo", tag="wo")
            x_sb = x_tiles
            wq_sb = [wq_all[:, t, :] for t in range(CT)]
            wo_sb = [wo_all[:, t, :] for t in range(CT)]
            q_sb = [pp.tile([128, LQ], BF16, name=f"q{t}", tag=f"q{t}") for t in range(CT)]
            k_sb = [pp.tile([128, L], BF16, name=f"k{t}", tag=f"k{t}") for t in range(CT)]
            # vt_sb[jp][t, h, s, d]: V^T in fp8 for DoubleRow: s in {0,1} picks
            # key chunk 2*jp / 2*jp+1; col 64 of each (h, s) block is the ones
            # column producing the softmax denominator on PSUM partition 64.
            vt_sb = [
                pp.tile([128, H, 2, D1], FP8, name=f"vt{j}", tag=f"vt{j}")
                for j in range(NJP)
            ]
            oh_all = pp.tile([128, CT, LQ], BF16, name="oh", tag="oh")
            nx_all = pp.tile([128, CT, L], BF16, name="nx", tag="nx")
            oh_sb = [oh_all[:, t, :] for t in range(CT)]
            nx_sb = [nx_all[:, t, :] for t in range(CT)]
            y_all = pp.tile([128, CT, LQ], BF16, name="y", tag="y")
            gnw_sb = pp.tile([CT, 128], F32, name="gnw", tag="gnw")
            gnb_sb = pp.tile([CT, 128], F32, name="gnb", tag="gnb")
            bout_sb = pp.tile([128, CT], F32, name="bout", tag="bout")
            ident_sb = pp.tile([128, 128], F32, name="ident", tag="ident")
            sparam_sb = pp.tile([128, 2, CT], F32, name="sparam", tag="sparam")
            ones64_sb = pp.tile([1, D], BF16, name="ones64", tag="ones64")
            ebias_sb = pp.tile([128, 1], F32, name="ebias", tag="ebias")

            # One DMA per 128-row tile: HWDGE charges ~500ns fixed per DMACopy
            # on a single shared device, so few big transfers beat many small
            # ones.  All startup-critical loads go on the SP queue (keeping the
            # ACT engine free for its stats-accumulate role and the exp
            # stream); wo/bout are only needed at out-proj time and ride the
            # Pool SWDGE path, which bypasses HWDGE entirely.
            # x3 third so its ACT-side stats overlap the DVE bn_stats of t0-t2
            for t in (0, 1, 3, 2):
                nc.sync.dma_start(x_sb[t][:], xd[128 * t : 128 * t + 128, :])
            nc.sync.dma_start(ident_sb[:], identd[:])
            nc.sync.dma_start(gnw_sb[:], gnwd[:])
            nc.sync.dma_start(gnb_sb[:], gnbd[:])
            for t in range(CT):
                nc.sync.dma_start(wq_all[:, t, :], wqkvT[128 * t : 128 * t + 128, :])
            nc.vector.memset(ones64_sb[:], 1.0)
            nc.vector.memset(ebias_sb[:], EXP_BIAS)

            # ---------------- group norm statistics ----------------
            with (
                tc.tile_pool(name="gtmp", bufs=2) as gp,
                tc.tile_pool(name="gps", bufs=2, space="PSUM") as gpp,
            ):
                # stats_all col t = channel-mean(tile t), col 32+t = channel-var:
                # after PE transpose, means land on partitions 0..3 and vars on
                # 32..35 (engine APs may only start at partition 0/32/64/96).
                stats_all = gp.tile([128, 36], F32, name="stats_all", tag="stats_all")
                nc.vector.memset(stats_all[:], 0.0)
                # tiles 0..2: DVE bn_stats; tile 3: ACT accumulate (Copy -> sum,
                # Square -> sum of squares; both live in the Exp table set so
                # the attention stream's table never switches).  The ACT main
                # outputs scribble into nx_sb[3], which the gn apply rewrites.
                for t in range(CT - 1):
                    st6 = gp.tile([128, 4, 6], F32, name="st6", tag="st6")
                    for sg in range(4):
                        nc.vector.bn_stats(
                            out=st6[:, sg, :],
                            in_=x_sb[t][:, 512 * sg : 512 * sg + 512],
                        )
                    sa = stats_all[:]
                    mv_out = bass.AP(
                        tensor=sa.tensor, offset=sa.offset + t, ap=[sa.ap[0], [32, 2]]
                    )
                    nc.vector.bn_aggr(out=mv_out, in_=st6[:])
                t3 = CT - 1
                sum3 = gp.tile([128, 1], F32, name="sum3", tag="sum3")
                ss3 = gp.tile([128, 1], F32, name="ss3", tag="ss3")
                with nc.allow_low_precision(reason="scratch output, accum is f32"):
                    nc.scalar.activation(
                        out=nx_sb[t3][:], in_=x_sb[t3][:], func=ACTF.Copy,
                        accum_out=sum3[:],
                    )
                    nc.scalar.activation(
                        out=nx_sb[t3][:], in_=x_sb[t3][:], func=ACTF.Square,
                        accum_out=ss3[:],
                    )
                m3tmp = gp.tile([128, 1], F32, name="m3tmp", tag="m3tmp")
                nc.vector.tensor_scalar(
                    out=stats_all[:, t3 : t3 + 1], in0=sum3[:],
                    scalar1=1.0 / L, op0=OP.mult, scalar2=0.0, op1=OP.add,
                )
                nc.vector.tensor_mul(
                    m3tmp[:], stats_all[:, t3 : t3 + 1], stats_all[:, t3 : t3 + 1]
                )
                nc.vector.scalar_tensor_tensor(
                    out=stats_all[:, 32 + t3 : 33 + t3],
                    in0=ss3[:],
                    scalar=1.0 / L,
                    in1=m3tmp[:],
                    op0=OP.mult,
                    op1=OP.subtract,
                )

                st_ps = gpp.tile([36, 128], F32, name="st_ps", tag="st_ps")
                nc.tensor.transpose(st_ps[:], stats_all[:], ident_sb[:])
                statsT = gp.tile([36, 128], F32, name="statsT", tag="statsT")
                nc.vector.tensor_copy(statsT[:], st_ps[:])

                mred = gp.tile([4, 8], F32, name="mred", tag="mred")
                nc.vector.tensor_reduce(
                    out=mred[:],
                    in_=statsT[0:4, :].rearrange("p (g s) -> p g s", s=16),
                    axis=AX.X,
                    op=OP.add,
                )
                vred = gp.tile([4, 8], F32, name="vred", tag="vred")
                nc.vector.tensor_reduce(
                    out=vred[:],
                    in_=statsT[32:36, :].rearrange("p (g s) -> p g s", s=16),
                    axis=AX.X,
                    op=OP.add,
                )
                sq = gp.tile([4, 128], F32, name="sq", tag="sq")
                nc.vector.tensor_mul(sq[:], statsT[0:4, :], statsT[0:4, :])
                sqred = gp.tile([4, 8], F32, name="sqred", tag="sqred")
                nc.vector.tensor_reduce(
                    out=sqred[:],
                    in_=sq[:].rearrange("p (g s) -> p g s", s=16),
                    axis=AX.X,
                    op=OP.add,
                )
                mg = gp.tile([4, 8], F32, name="mg", tag="mg")
                nc.vector.tensor_scalar_mul(mg[:], mred[:], 1.0 / 16)
                # vg = red_var/16 + sqred/16 - mg^2
                vg = gp.tile([4, 8], F32, name="vg", tag="vg")
                nc.vector.tensor_scalar_mul(vg[:], vred[:], 1.0 / 16)
                nc.vector.scalar_tensor_tensor(
                    out=vg[:],
                    in0=sqred[:],
                    scalar=1.0 / 16,
                    in1=vg[:],
                    op0=OP.mult,
                    op1=OP.add,
                )
                mg2 = gp.tile([4, 8], F32, name="mg2", tag="mg2")
                nc.vector.tensor_mul(mg2[:], mg[:], mg[:])
                nc.vector.tensor_sub(vg[:], vg[:], mg2[:])
                # rstd = rsqrt(vg + eps) via DVE-only Newton iteration (y0 = 1:
                # group variances of normalized-scale data sit near 1, and each
                # iteration squares the relative error).  Keeps the ACT engine's
                # table slot pinned to Exp for the attention stream.
                nc.vector.tensor_scalar(
                    out=vg[:], in0=vg[:], scalar1=EPS, op0=OP.add,
                    scalar2=0.0, op1=OP.add,
                )
                ny = gp.tile([4, 8], F32, name="ny", tag="ny")
                nt = gp.tile([4, 8], F32, name="nt", tag="nt")
                nc.vector.memset(ny[:], 1.0)
                for _ in range(3):
                    nc.vector.tensor_mul(nt[:], ny[:], ny[:])
                    nc.vector.tensor_mul(nt[:], nt[:], vg[:])
                    nc.vector.tensor_scalar(
                        out=nt[:], in0=nt[:], scalar1=-0.5, op0=OP.mult,
                        scalar2=1.5, op1=OP.add,
                    )
                    nc.vector.tensor_mul(ny[:], ny[:], nt[:])
                nc.vector.tensor_copy(vg[:], ny[:])

                # broadcast group -> channels: [4, 8] -> [4, 128]
                def bcast16(src):
                    a = src.ap
                    return bass.AP(
                        tensor=src.tensor, offset=src.offset, ap=[a[0], a[1], [0, 16]]
                    )

                rstd_bc = gp.tile([4, 128], F32, name="rstd_bc", tag="rstd_bc")
                nc.vector.tensor_copy(
                    rstd_bc[:].rearrange("p (g s) -> p g s", s=16), bcast16(vg[:])
                )
                mg_bc = gp.tile([4, 128], F32, name="mg_bc", tag="mg_bc")
                nc.vector.tensor_copy(
                    mg_bc[:].rearrange("p (g s) -> p g s", s=16), bcast16(mg[:])
                )
                s2 = gp.tile([4, 128], F32, name="s2", tag="s2")
                nc.vector.tensor_mul(s2[:], rstd_bc[:], gnw_sb[0:4, :])
                s1 = gp.tile([4, 128], F32, name="s1", tag="s1")
                nc.vector.reciprocal(out=s1[:], in_=s2[:])
                nc.vector.tensor_mul(s1[:], s1[:], gnb_sb[0:4, :])
                nc.vector.tensor_sub(s1[:], mg_bc[:], s1[:])

                sp_ps = gpp.tile([128, 2, CT], F32, name="sp_ps", tag="sp_ps")
                nc.tensor.transpose(sp_ps[:, 0, :], s1[:], ident_sb[0:4, 0:4])
                nc.tensor.transpose(sp_ps[:, 1, :], s2[:], ident_sb[0:4, 0:4])
                nc.vector.tensor_copy(sparam_sb[:], sp_ps[:])

            # group-norm apply: nx = (x - s1) * s2, cast to fp8 for the
            # DoubleRow qkv matmuls.  Emitted nb-major so the first qkv
            # matmuls (which need all 4 channel tiles of one 512-column
            # block) unblock early.
            with nc.allow_low_precision(reason="fp8 qkv inputs intended"):
                for nb in range(4):
                    for t in range(CT):
                        sl = slice(512 * nb, 512 * nb + 512)
                        nc.vector.tensor_scalar(
                            out=nx_sb[t][:, sl],
                            in0=x_sb[t][:, sl],
                            scalar1=sparam_sb[:, 0, t : t + 1],
                            scalar2=sparam_sb[:, 1, t : t + 1],
                            op0=OP.subtract,
                            op1=OP.mult,
                        )

            # out-proj weights load behind everything else on the same SP
            # queue — strictly after x/wq in HWDGE order, and only needed at
            # proj time.
            for t in range(CT):
                nc.sync.dma_start(wo_all[:, t, :], woutT[128 * t : 128 * t + 128, :])
            nc.sync.dma_start(bout_sb[:], boutd[:])

            # ---------------- pipelined qkv + attention + proj ----------------
            with (
                tc.tile_pool(name="psS", bufs=1, space="PSUM") as pS,
                tc.tile_pool(name="psO", bufs=1, space="PSUM") as pO,
                tc.tile_pool(name="psU", bufs=2, space="PSUM") as pU,
                tc.tile_pool(name="expp", bufs=4) as ep,
                tc.tile_pool(name="rcpp", bufs=4) as rp,
            ):
                o_tiles = [
                    pO.tile([128, 512], F32, name=f"O{h01}", tag=f"O{h01}")
                    for h01 in range(2)
                ]

                # ---- PE filler work units (qkv / proj), emitted into stream slack
                def qkv_unit(kind, t, nb):
                    """One psum tile of q/k production + its PSUM->SBUF copy."""
                    ps = pU.tile([128, 512], F32, name="qkU", tag="qkU")
                    off = 0 if kind == "q" else C
                    for c in range(CT):
                        nc.tensor.matmul(
                            ps[:],
                            wq_all[:, c, off + 128 * t : off + 128 * t + 128],
                            nx_all[:, c, 512 * nb : 512 * nb + 512],
                            start=(c == 0),
                            stop=(c == CT - 1),
                        )
                    dst = q_sb[t] if kind == "q" else k_sb[t]
                    nc.vector.tensor_copy(dst[:, 512 * nb : 512 * nb + 512], ps[:])

                def vt_unit(jc):
                    """One key-chunk of V^T -> fp8 vt tile (s = jc parity)."""
                    jp, s = jc // 2, jc % 2
                    ps = pU.tile([128, 512], F32, name="qkU", tag="qkU")
                    for c in range(CT):
                        nc.tensor.matmul(
                            ps[:],
                            nx_all[:, c, 128 * jc : 128 * jc + 128],
                            wq_all[:, c, 2 * C : 3 * C],
                            start=(c == 0),
                            stop=(c == CT - 1),
                        )
                    with nc.allow_low_precision(reason="fp8 attention V intended"):
                        nc.vector.tensor_copy(
                            vt_sb[jp][:, :, s, 0:D],
                            ps[:].rearrange("p (h d) -> p h d", d=D),
                        )
                    nc.vector.memset(vt_sb[jp][:, :, s, D : D1], 0.0)
                    nc.vector.memset(vt_sb[jp][:, :, s, D : D + 1], 1.0)

                def proj_unit(t, ib):
                    """One out-proj tile + bias + residual + store."""
                    ps = pU.tile([128, 512], F32, name="qkU", tag="qkU")
                    for c in range(CT):
                        nc.tensor.matmul(
                            ps[:],
                            wo_all[:, c, 128 * t : 128 * t + 128],
                            oh_all[:, c, 512 * ib : 512 * ib + 512],
                            start=(c == 0),
                            stop=(c == CT - 1),
                        )
                    sl = slice(512 * ib, 512 * ib + 512)
                    nc.vector.scalar_tensor_tensor(
                        out=y_all[:, t, sl],
                        in0=ps[:],
                        scalar=bout_sb[:, t : t + 1],
                        in1=x_sb[t][:, sl],
                        op0=OP.add,
                        op1=OP.add,
                    )
                    (nc.sync if (t + ib) % 2 else nc.scalar).dma_start(
                        yd[128 * t : 128 * t + 128, sl], y_all[:, t, sl]
                    )

                # filler schedule: fillers[u] = list of thunks emitted after unit u.
                # Unit u = (pair, ib, jp, h01) with pair outer (u // 32), then
                # ib (u // 16 % 2), jp (u // 2 % 8), h01 (u % 2).
                NU = 2 * H * NIB * 1 * NJP // 2  # = 128 units
                fillers = [[] for _ in range(NU + 3)]

                def sched(u, fn, *a):
                    fillers[min(u, NU + 2)].append((fn, a))

                # prelude (emitted before the stream): k0/q0 for the first
                # S-matmuls and vt0 for the first O.
                prelude = [
                    (qkv_unit, ("k", 0, 0)),
                    (qkv_unit, ("q", 0, 0)),
                    (vt_unit, (0,)),
                    (vt_unit, (1,)),
                ]
                # pair0-ib0 (units 0..15) consumes all of k0 and vt0..15.
                # Deadlines: S(u) needs k cols for jp=(u//2)%8 before emission
                # step u; O(u) (emitted at step u+2) needs vt[jp] complete.
                sched(0, qkv_unit, "k", 0, 1)
                sched(0, vt_unit, 2)
                sched(1, vt_unit, 3)
                sched(2, vt_unit, 4)
                sched(3, vt_unit, 5)
                sched(4, vt_unit, 6)
                sched(4, qkv_unit, "k", 0, 2)
                sched(5, vt_unit, 7)
                sched(6, vt_unit, 8)
                sched(7, vt_unit, 9)
                sched(7, qkv_unit, "k", 0, 3)
                sched(8, vt_unit, 10)
                sched(9, vt_unit, 11)
                sched(10, vt_unit, 12)
                sched(11, vt_unit, 13)
                sched(12, vt_unit, 14)
                sched(13, vt_unit, 15)
                sched(14, qkv_unit, "q", 0, 1)
                # Remaining qkv spread as evenly as deadlines allow, so the PE
                # never idles long (TRN2 drops the tensor-engine pstate when
                # idle, which would slow the latency-critical S matmuls).
                for i in range(4):
                    sched(18 + 2 * i, qkv_unit, "k", 1, i)
                sched(26, qkv_unit, "q", 1, 0)
                sched(28, qkv_unit, "q", 1, 1)
                for i in range(4):
                    sched(34 + 5 * i, qkv_unit, "k", 2, i)
                sched(56, qkv_unit, "q", 2, 0)
                sched(60, qkv_unit, "q", 2, 1)
                for i in range(4):
                    sched(66 + 5 * i, qkv_unit, "k", 3, i)
                sched(88, qkv_unit, "q", 3, 0)
                sched(92, qkv_unit, "q", 3, 1)
                # pair3-ib1 (112..127): proj of ib0; tail: proj of ib1
                for t in range(CT):
                    sched(114 + 3 * t, proj_unit, t, 0)
                for t in range(CT):
                    sched(NU + 2, proj_unit, t, 1)

                for fn, a in prelude:
                    fn(*a)

                def unit_of(u):
                    pair = u // 32
                    ib = (u // 16) % 2
                    jp = (u // 2) % 8
                    h01 = u % 2
                    return pair, ib, jp, h01

                s_slots = [
                    pS.tile([128, 2, 512], F32, name=f"S{i}", tag=f"S{i}")
                    for i in range(2)
                ]
                et_live = {}   # u -> et tile
                s_live = {}    # u -> psum slot

                def emit_S(u):
                    pair, ib, jp, h01 = unit_of(u)
                    slot = s_slots[u % 2]
                    s_live[u] = slot
                    kt, qt = k_sb[pair], q_sb[pair]
                    qh = qt[64 * h01 : 64 * h01 + 64, 512 * ib : 512 * ib + 512]
                    for s in range(2):
                        jc = 2 * jp + s
                        nc.tensor.matmul(
                            slot[:, s, :],
                            kt[64 * h01 : 64 * h01 + 64, 128 * jc : 128 * jc + 128],
                            qh,
                            start=True,
                            stop=True,
                        )

                def emit_exp(u):
                    et = ep.tile([128, 2, 512], FP8, name="et", tag="et")
                    et_live[u] = et
                    with nc.allow_low_precision(reason="fp8 softmax weights intended"):
                        nc.scalar.activation(
                            out=et[:],
                            in_=s_live.pop(u)[:],
                            func=ACTF.Exp,
                            scale=float(D) ** -0.5,
                            bias=ebias_sb[:],
                        )

                def emit_O(u):
                    pair, ib, jp, h01 = unit_of(u)
                    ot = o_tiles[h01]
                    nc.tensor.matmul(
                        ot[0:D1, :],
                        vt_sb[jp][:, 2 * pair + h01, :, :],
                        et_live.pop(u)[:],
                        start=(jp == 0),
                        stop=(jp == NJP - 1),
                        perf_mode=DR,
                    )
                    if jp == NJP - 1:
                        # normalize: oh = O[0:64] * (1 / denom-row-64).  Copy
                        # the accumulator to SBUF first so the PSUM bank frees
                        # for the next (pair, ib) immediately; broadcast the
                        # reciprocal row across partitions on the Pool engine.
                        ocp = rp.tile([65, 512], F32, name="ocp", tag="ocp")
                        nc.vector.tensor_copy(ocp[:], ot[0:65, :])
                        rcp = rp.tile([1, 512], BF16, name="rcp", tag="rcp")
                        with nc.allow_low_precision(reason="bf16 softmax recip intended"):
                            nc.vector.reciprocal(out=rcp[:], in_=ocp[64:65, :])
                        bc = pU.tile([128, 512], F32, name="qkU", tag="qkU")
                        nc.tensor.matmul(
                            bc[0:64, :],
                            ones64_sb[:],
                            rcp[:],
                            start=True,
                            stop=True,
                        )
                        nc.vector.tensor_mul(
                            oh_sb[pair][
                                64 * h01 : 64 * h01 + 64, 512 * ib : 512 * ib + 512
                            ],
                            ocp[0:64, :],
                            bc[0:64, :],
                        )

                for n in range(NU + 3):
                    if n < NU:
                        emit_S(n)
                    if 1 <= n and n - 1 < NU:
                        emit_exp(n - 1)
                    if 2 <= n and n - 2 < NU:
                        emit_O(n - 2)
                    for fn, a in fillers[n] if n < len(fillers) else []:
                        fn(*a)

    nc.compile()
    return nc


_NC_CACHE = None


def _get_nc():
    global _NC_CACHE
    if _NC_CACHE is None:
        _NC_CACHE = build_nc()
    return _NC_CACHE


def _host_inputs(x, gn_w, gn_b, w_qkv, w_out, b_out):
    w_qkvT = np.ascontiguousarray(w_qkv.T).astype(ml_dtypes.bfloat16)
    w_outT = np.ascontiguousarray(w_out.T).astype(ml_dtypes.bfloat16)
    ident = np.eye(128, dtype=np.float32)
    shared = {
        "wqkvT": w_qkvT,
        "woutT": w_outT,
        "gnw": np.ascontiguousarray(gn_w.reshape(CT, 128), np.float32),
        "gnb": np.ascontiguousarray(gn_b.reshape(CT, 128), np.float32),
        "bout": np.ascontiguousarray(b_out.reshape(CT, 128).T, np.float32),
        "ident": ident,
    }
    in_maps = []
    for core in range(8):
        b, ih = core // 2, core % 2
        xb = np.asarray(x[b], np.float32)
        if ih:
            xb = np.concatenate([xb[:, LQ:], xb[:, :LQ]], axis=1)
        in_maps.append(
            {"x": np.ascontiguousarray(xb).astype(ml_dtypes.bfloat16), **shared}
        )
    return in_maps


def kernel(x, gn_w, gn_b, w_qkv, w_out, b_out):
    nc = _get_nc()
    in_maps = _host_inputs(
        np.asarray(x), np.asarray(gn_w), np.asarray(gn_b),
        np.asarray(w_qkv), np.asarray(w_out), np.asarray(b_out),
    )
    res = run_bass_kernel_spmd(nc, in_maps, list(range(8)))
    y = np.empty((B, C, L), np.float32)
    for core in range(8):
        b, ih = core // 2, core % 2
        y[b][:, ih * LQ : (ih + 1) * LQ] = res.results[core]["y"].astype(np.float32)
    return y

